# revision 38
# baseline (speedup 1.0000x reference)
"""Trainium2 Bass kernel for a 3-layer GCN (JKNet, mode='cat') — 8-core SPMD.

Strategy (dst-sharded graph parallelism):
  - Nodes are partitioned across 8 cores (6250 each, padded to 6272 = 49*128).
    Each core owns all edges whose destination lands in its range.
  - Per layer: each core computes its slice of h @ W, an AllGather builds the
    full 50176x64 f32 gather table in DRAM, then the core gathers hW[src] for
    its edges with GPSIMD dma_gather (4 SWDGE queues), scales by edge weight
    on DVE and accumulates into an SBUF accumulator using a "rounds" schedule:
    nodes are degree-sorted so round s covers a dense prefix of node slots,
    making the accumulate a plain strided DVE add (no scatter anywhere).
  - dma_gather indices are int16, so the table is split in two halves
    (rows < 25088 / >= 25088, i.e. src cores 0-3 vs 4-7). Each half gets its
    own degree-sort permutation and accumulator; the second accumulator is
    reconciled through a DRAM bounce + local permute-gather.
  - Large gather chunks (4096 rows) amortize the ~1us fixed SWDGE cost per
    dma_gather instruction. Messages are scaled into fp16 and accumulated in
    fp16 (2x DVE rate); the gather table stays f32 (256B-row requirement).
  - Bias + ReLU on DVE/ACT, PE transpose (fp16) produces h^T for the next
    layer's matmul and for the final JumpingKnowledge concat matmul.

Self-contained: hardcodes the problem geometry (N=50000, E=800000, 128->64,
3 layers, out 40) but computes all data-dependent schedules from the inputs.
"""

import sys

sys.path.insert(0, "/opt/trn_rl_repo")

import numpy as np

N = 50000
E = 800000
IN_DIM = 128
HID = 64
OUT_DIM = 40
M = 8               # cores
NPC = N // M        # 6250 nodes per core
SLOTS = 49          # ceil(6250/128)
SLICE = SLOTS * 128  # 6272 padded rows per core slice
TABLE_ROWS = M * SLICE  # 50176
HALF = 4 * SLICE    # 25088 (int16-safe boundary; src cores 0-3 vs 4-7)
CMAX = 896          # max indices per dma_gather instruction
NQ = 4              # SWDGE queues


def _wrap16(a):
    """Flat [L] -> [128, L//16] int16, index j at partition j%16, slot j//16,
    replicated across the 8 GPSIMD core groups."""
    L = a.shape[0]
    return np.tile(a.reshape(L // 16, 16).T, (8, 1)).astype(np.int16)


def _wrap128(a):
    """Flat [L] -> [128, L//128], position j at partition j%128, slot j//128."""
    L = a.shape[0]
    return np.ascontiguousarray(a.reshape(L // 128, 128).T)


def _rowof(q):
    """acc position q -> wrapped DRAM row index (partition-major layout)."""
    return (q % 128) * SLOTS + q // 128


def _ranks_within(p):
    """For int array p, rank of each element among equal values (stable)."""
    order = np.argsort(p, kind="stable")
    ps = p[order]
    starts = np.r_[0, np.nonzero(np.diff(ps))[0] + 1]
    counts = np.diff(np.r_[starts, len(ps)])
    r_sorted = np.arange(len(ps)) - np.repeat(starts, counts)
    r = np.empty_like(r_sorted)
    r[order] = r_sorted
    return r


def _build_system(pos_of_dst, table_row_of_src, ew, max_rounds_widths, chunks):
    """Build flat (idx, ew) arrays for one round-system of one core.

    pos_of_dst: per-edge acc position of the destination node (degree-sorted).
    table_row_of_src: per-edge gather index (already half-relative).
    max_rounds_widths: global per-round padded widths W_s (list, multiple of 128).
    chunks: output of _chunk_plan; positions past each chunk's real count get
    index -1 so the gather ucode skips them (num_idxs_reg).
    Returns (idx_flat int64, ew_flat f32).
    """
    roundoff = np.r_[0, np.cumsum(max_rounds_widths)]
    L = int(roundoff[-1])
    idx_flat = np.zeros(L, np.int64)
    ew_flat = np.zeros(L, np.float32)
    r = _ranks_within(pos_of_dst)
    flatpos = roundoff[r] + pos_of_dst
    idx_flat[flatpos] = table_row_of_src
    ew_flat[flatpos] = ew
    for (off, w, _segs, cnt) in chunks:
        idx_flat[off + cnt:off + w] = -1
    return idx_flat, ew_flat


def _plan_rounds(deg_by_core):
    """deg_by_core: [M, NPC] degree of each node (in its own sort order, desc).
    Returns (padded round widths, real max counts), widths multiple of 128."""
    smax = int(max(d[0] for d in deg_by_core)) if len(deg_by_core) else 0
    widths, reals = [], []
    for s in range(smax):
        n_s = max(int((d > s).sum()) for d in deg_by_core)
        if n_s == 0:
            break
        widths.append(((n_s + 127) // 128) * 128)
        reals.append(n_s)
    return widths, reals


def _chunk_plan(widths, reals, total_pad128):
    """Split the concatenated padded rounds into gather chunks of at most
    CMAX, never crossing a round boundary (so per-round trailing padding is
    trailing within its chunk and can be skipped via num_idxs_reg).
    Returns list of (off, w, [(msg_slot0, acc_slot0, nslots)], real_cnt)."""
    roundoff = np.r_[0, np.cumsum(widths)].astype(np.int64)
    assert int(roundoff[-1]) == total_pad128
    chunks = []
    for s, w_s in enumerate(widths):
        start = int(roundoff[s])
        off = start
        while off < start + w_s:
            w = min(CMAX, start + w_s - off)
            cnt = max(0, min(reals[s] - (off - start), w))
            assert cnt > 0
            chunks.append((off, w, [(0, (off - start) // 128, w // 128)], cnt))
            off += w
    return chunks


def _prep(x, edge_index, edge_weight):
    """All host-side index prep. Returns (plan dict, per-core input maps)."""
    src = np.asarray(edge_index[0], dtype=np.int64)
    dst = np.asarray(edge_index[1], dtype=np.int64)
    ew = np.asarray(edge_weight, dtype=np.float32)
    x = np.asarray(x, dtype=np.float32)

    dcore = dst // NPC
    dloc = dst - dcore * NPC
    score = src // NPC
    is_own = np.zeros_like(score, dtype=bool)
    is_a = (score <= 3) & ~is_own
    is_b = (score > 3) & ~is_own

    # per-core degree sorts for the three systems
    posA = np.empty(N, np.int64)   # node -> acc/table position (A order)
    posB = np.empty(N, np.int64)
    posO = np.empty(N, np.int64)
    piA_all = []
    degA_sorted, degB_sorted, degO_sorted = [], [], []
    for c in range(M):
        mask = dcore == c
        la = dloc[mask & is_a]
        lb = dloc[mask & is_b]
        lo = dloc[mask & is_own]
        degA = np.bincount(la, minlength=NPC)
        degB = np.bincount(lb, minlength=NPC)
        degO = np.bincount(lo, minlength=NPC)
        piA = np.argsort(-degA, kind="stable")
        piB = np.argsort(-degB, kind="stable")
        piO = np.argsort(-degO, kind="stable")
        pA = np.empty(NPC, np.int64); pA[piA] = np.arange(NPC)
        pB = np.empty(NPC, np.int64); pB[piB] = np.arange(NPC)
        pO = np.empty(NPC, np.int64); pO[piO] = np.arange(NPC)
        posA[c * NPC:(c + 1) * NPC] = pA
        posB[c * NPC:(c + 1) * NPC] = pB
        posO[c * NPC:(c + 1) * NPC] = pO
        piA_all.append(piA)
        degA_sorted.append(degA[piA])
        degB_sorted.append(degB[piB])
        degO_sorted.append(degO[piO])

    widthsA, realsA = _plan_rounds(degA_sorted)
    widthsB, realsB = _plan_rounds(degB_sorted)
    LA = int(np.sum(widthsA))
    LB = int(np.sum(widthsB))
    LO = 0
    chunksA = _chunk_plan(widthsA, realsA, LA)
    chunksB = _chunk_plan(widthsB, realsB, LB)
    chunksO = []

    # global table row of a node (wrapped within its owner's slice)
    table_row = (np.arange(N) // NPC) * SLICE + _rowof(posA)

    in_maps = []
    for c in range(M):
        mask = dcore == c
        mA = mask & is_a
        mB = mask & is_b
        mO = mask & is_own
        idxA, ewA = _build_system(posA[dst[mA]] , table_row[src[mA]], ew[mA], widthsA, chunksA)
        idxB, ewB = _build_system(posB[dst[mB]], table_row[src[mB]] - HALF, ew[mB], widthsB, chunksB)
        assert idxA.max(initial=0) < HALF and idxB.max(initial=0) < HALF

        # permute maps: A-position q -> wrapped bounce row of the same node's
        # B/O-position. Pad positions (>= NPC) point at an always-zero row.
        piA = piA_all[c]
        rho = np.full(SLICE, NPC, np.int64)
        rho[:NPC] = posB[c * NPC + piA]
        rho_rows = _rowof(rho)

        # x slice, transposed, in A order (pad columns zero)
        xT = np.zeros((IN_DIM, SLICE), np.float16)
        xT[:, :NPC] = x[c * NPC + piA, :].T

        in_maps.append({
            "xT": xT,
            "idxA": _wrap16(idxA), "ewA": _wrap128(ewA),
            "idxB": _wrap16(idxB), "ewB": _wrap128(ewB),
            "rho": _wrap16(rho_rows),
        })

    plan = {
        "LA": LA, "LB": LB, "LO": LO,
        "chunksA": chunksA, "chunksB": chunksB, "chunksO": chunksO,
        "widthsA": widthsA,
        "posA": posA,
    }
    return plan, in_maps


def _build(plan, W1, b1, W2, b2, W3, b3, Wlin, blin):
    import concourse.bacc as bacc
    import concourse.mybir as mybir
    import concourse.tile as tile

    LA, LB, LO = plan["LA"], plan["LB"], plan["LO"]
    f32 = mybir.dt.float32
    f16 = mybir.dt.float16
    i16 = mybir.dt.int16

    nc = bacc.Bacc("TRN2", target_bir_lowering=False, debug=False,
                   num_devices=M, num_swdge_queues=NQ)

    # ---- I/O ----
    xT_d = nc.dram_tensor("xT", [IN_DIM, SLICE], f16, kind="ExternalInput")
    idxA_d = nc.dram_tensor("idxA", [128, LA // 16], i16, kind="ExternalInput")
    ewA_d = nc.dram_tensor("ewA", [128, LA // 128], f32, kind="ExternalInput")
    idxB_d = nc.dram_tensor("idxB", [128, LB // 16], i16, kind="ExternalInput")
    ewB_d = nc.dram_tensor("ewB", [128, LB // 128], f32, kind="ExternalInput")
    rho_d = nc.dram_tensor("rho", [128, SLICE // 16], i16, kind="ExternalInput")

    W1_d = nc.dram_tensor("W1", [IN_DIM, HID], f16, kind="ExternalInput")
    W2_d = nc.dram_tensor("W2", [HID, HID], f16, kind="ExternalInput")
    W3_d = nc.dram_tensor("W3", [128, HID], f16, kind="ExternalInput")  # rows 64-127 hold W3
    Wl12_d = nc.dram_tensor("Wl12", [128, OUT_DIM], f16, kind="ExternalInput")
    Wl3_d = nc.dram_tensor("Wl3", [HID, OUT_DIM], f16, kind="ExternalInput")
    bias_d = nc.dram_tensor("bias", [128, 3 * HID], f16, kind="ExternalInput")
    blin_d = nc.dram_tensor("blin", [128, OUT_DIM], f32, kind="ExternalInput")
    out_d = nc.dram_tensor("out", [128, SLOTS, OUT_DIM], f32, kind="ExternalOutput")

    # internal DRAM
    slice_d = nc.dram_tensor("slice_hw", [128, SLOTS, HID], f32)
    table_d = nc.dram_tensor("table", [TABLE_ROWS, HID], f32, addr_space="Shared")
    bounce_d = nc.dram_tensor("bounce", [SLICE, HID], f32)


    qctr = [0]

    def nextq():
        q = qctr[0] % NQ
        qctr[0] += 1
        return q

    with tile.TileContext(nc) as tc:
        with (
            tc.tile_pool(name="const", bufs=1) as constp,
            tc.tile_pool(name="acc", bufs=1) as accp,
            tc.tile_pool(name="ht", bufs=1) as htp,
            tc.tile_pool(name="stag", bufs=1) as stagp,
            tc.tile_pool(name="msg", bufs=28) as msgp,
            tc.tile_pool(name="msgw", bufs=28) as msgwp,
            tc.tile_pool(name="warm", bufs=NQ) as warmp,
            tc.tile_pool(name="ps", bufs=3, space="PSUM") as psp,
            tc.tile_pool(name="pso", bufs=2, space="PSUM") as psop,
        ):
            # ---- load constants ----
            xT = constp.tile([IN_DIM, SLICE], f16)
            idxA = constp.tile([128, LA // 16], i16)
            ewA = constp.tile([128, LA // 128], f32)
            idxB = constp.tile([128, LB // 16], i16)
            ewB = constp.tile([128, LB // 128], f32)
            rho = constp.tile([128, SLICE // 16], i16)

            W1t = constp.tile([IN_DIM, HID], f16)
            W2t = constp.tile([HID, HID], f16)
            W3t = constp.tile([128, HID], f16)  # W3 lives in partitions 64-127
            Wl12t = constp.tile([128, OUT_DIM], f16)
            Wl3t = constp.tile([HID, OUT_DIM], f16)
            biast = constp.tile([128, 3 * HID], f16)
            blint = constp.tile([128, OUT_DIM], f32)
            ident = constp.tile([128, 128], f16)

            for k in range(0, SLOTS, 7):
                cs = slice(k * 128, (k + 7) * 128)
                nc.sync.dma_start(xT[:, cs], xT_d[:, cs])
            for t, d in ((idxA, idxA_d), (ewA, ewA_d),
                         (idxB, idxB_d), (ewB, ewB_d), (rho, rho_d),
                         (W1t, W1_d), (W2t, W2_d), (Wl12t, Wl12_d),
                         (Wl3t, Wl3_d),
                         (biast, bias_d), (blint, blin_d)):
                nc.sync.dma_start(t[:], d[:])
            nc.sync.dma_start(W3t[:], W3_d[:])
            from concourse.masks import make_identity
            make_identity(nc, ident[:])

            for _ in range(28):
                wt = msgp.tile([128, CMAX // 128, HID], f32, tag="msg")
                nc.vector.memset(wt[:], 0.0)

            h12T = htp.tile([128, SLICE], f16)   # rows 0-63: h1^T, 64-127: h2^T
            h3T = htp.tile([HID, SLICE], f16)

            relu = mybir.ActivationFunctionType.Relu
            copyf = mybir.ActivationFunctionType.Copy
            rfull = nc.gpsimd.to_reg(CMAX)

            # ---- layer-1 input matmuls: slice of x @ W1 ----
            stag = stagp.tile([128, SLOTS, HID], f32, tag="stag")
            for m in range(SLOTS):
                ps = psp.tile([128, HID], f32, tag="mm")
                nc.tensor.matmul(ps[:], xT[:, m * 128:(m + 1) * 128], W1t[:],
                                 start=True, stop=True)
                nc.scalar.activation(stag[:, m, :], ps[:], copyf)
            nc.sync.dma_start(slice_d[:], stag[:])

            ostag = stagp.tile([128, SLOTS, OUT_DIM], f32, tag="ostag")

            for layer in range(3):
                # ---- AllGather the table for this layer ----
                nc.gpsimd.collective_compute(
                    "AllGather", mybir.AluOpType.bypass,
                    replica_groups=[list(range(M))],
                    ins=[slice_d[:]], outs=[table_d[:]],
                )

                accA = accp.tile([128, SLOTS, HID], f16, tag="accA")
                accB = accp.tile([128, SLOTS, HID], f16, tag="accB")
                nc.vector.memset(accA[:], 0.0)
                nc.vector.memset(accB[:], 0.0)
                if True:
                    # tiny warmup gathers on each queue while the AllGather
                    # runs, so post-collective DGE state reload happens off
                    # the critical path (results are discarded)
                    for _ in range(NQ):
                        wmsg = warmp.tile([128, 1, HID], f32, tag="wmsg")
                        nc.gpsimd.dma_gather(
                            wmsg[:, 0:1, :], bounce_d[:], rho[:, 0:8],
                            128, 128, HID, single_packet=False,
                            queue_num=nextq())

                if layer < 2:
                    stag = stagp.tile([128, SLOTS, HID], f32, tag="stag")

                def emit_chunks(acc, idx_t, ew_t, chunks, tbl, sp=False):
                    for (off, w, segs, cnt) in chunks:
                        ws = w // 128
                        msg = msgp.tile([128, CMAX // 128, HID], f32, tag="msg")
                        msgw = msgwp.tile([128, CMAX // 128, HID], f16, tag="msgw")
                        nreg = rfull if cnt == CMAX else cnt
                        nc.gpsimd.dma_gather(
                            msg[:, :ws, :], tbl, idx_t[:, off // 16:(off + w) // 16],
                            w, nreg, HID, single_packet=sp, queue_num=nextq())
                        nc.vector.tensor_mul(
                            msgw[:, :ws, :], msg[:, :ws, :],
                            ew_t[:, off // 128:(off + w) // 128]
                            .to_broadcast([128, ws, HID]))
                        for (ms, as_, ns) in segs:
                            nc.vector.tensor_add(
                                acc[:, as_:as_ + ns, :], acc[:, as_:as_ + ns, :],
                                msgw[:, ms:ms + ns, :])

                bslice = biast[:, layer * HID:(layer + 1) * HID]

                def emit_tail(slots_list):
                    # finalize the given acc slots: bias+relu, transpose into
                    # h^T, next-layer matmul, staging write. Runs on PE/ACT/
                    # DVE while GPSIMD keeps gathering remaining rounds.
                    slots_list = sorted(slots_list)
                    ranges = []
                    for m in slots_list:
                        if ranges and ranges[-1][1] == m:
                            ranges[-1][1] = m + 1
                        else:
                            ranges.append([m, m + 1])
                    for (m0, m1) in ranges:
                        r = m1 - m0
                        nc.vector.tensor_add(
                            accA[:, m0:m1, :], accA[:, m0:m1, :],
                            bslice.rearrange("p (s d) -> p s d", s=1)
                            .to_broadcast([128, r, HID]))
                        nc.scalar.activation(accA[:, m0:m1, :],
                                             accA[:, m0:m1, :], relu)
                    for (m0, m1) in ranges:
                        for m in range(m0, m1):
                            pst = psp.tile([HID, 128], f16, tag="tr")
                            nc.tensor.transpose(pst[:], accA[:, m, :], ident[:])
                            sl = slice(m * 128, (m + 1) * 128)
                            if layer == 0:
                                dst_ap = h12T[0:HID, sl]
                            elif layer == 1:
                                dst_ap = h12T[HID:128, sl]
                            else:
                                dst_ap = h3T[:, sl]
                            if m % 2 == 0:
                                nc.vector.tensor_copy(dst_ap, pst[:])
                            else:
                                nc.scalar.activation(dst_ap, pst[:], copyf)
                    for (m0, m1) in ranges:
                        for m in range(m0, m1):
                            sl = slice(m * 128, (m + 1) * 128)
                            if layer == 0:
                                ps = psp.tile([128, HID], f32, tag="mm")
                                nc.tensor.matmul(ps[:], h12T[0:HID, sl], W2t[:],
                                                 start=True, stop=True)
                                nc.scalar.activation(stag[:, m, :], ps[:], copyf)
                            elif layer == 1:
                                ps = psp.tile([128, HID], f32, tag="mm")
                                nc.tensor.matmul(ps[:], h12T[HID:128, sl],
                                                 W3t[HID:128, :],
                                                 start=True, stop=True)
                                nc.scalar.activation(stag[:, m, :], ps[:], copyf)
                            else:
                                pso = psop.tile([128, OUT_DIM], f32, tag="out")
                                nc.tensor.matmul(pso[:], h12T[:, sl],
                                                 Wl12t[:], start=True, stop=False)
                                nc.tensor.matmul(pso[:], h3T[:, sl],
                                                 Wl3t[:], start=False, stop=True)
                                nc.vector.tensor_add(ostag[:, m, :], pso[:],
                                                     blint[:])
                        if layer < 2:
                            nc.sync.dma_start(slice_d[:, m0:m1, :],
                                              stag[:, m0:m1, :])

                # per-slot finalization schedule: slot m is final after the
                # last A-chunk whose rounds still reach it (round widths
                # shrink, so high slots finalize early)
                import numpy as _np
                widthsA = plan["widthsA"]
                chA = plan["chunksA"]
                roundoffA = _np.r_[0, _np.cumsum(widthsA)].astype(_np.int64)
                NF = 14
                batches = {}
                for m in range(SLOTS):
                    ss = [s for s in range(len(widthsA))
                          if widthsA[s] > 128 * m]
                    send = int(roundoffA[max(ss) + 1]) - 1
                    for k, (off, w, _s, _c) in enumerate(chA):
                        if off <= send < off + w:
                            batches.setdefault(max(k, NF - 1), []).append(m)
                            break

                # B system first so its bounce DMA overlaps the A gathers
                emit_chunks(accB, idxB, ewB, plan["chunksB"],
                            table_d[HALF:TABLE_ROWS, :])
                # fp16 acc -> f32 on the idle ACT engine (gather needs 256B
                # rows), then a sync-engine DMA so GPSIMD is never blocked
                accB32 = accp.tile([128, SLOTS, HID], f32, tag="accB32")
                nc.scalar.activation(accB32[:], accB[:], copyf)
                nc.sync.dma_start(
                    bounce_d[:].rearrange("(p s) d -> p s d", p=128), accB32[:])
                emit_chunks(accA, idxA, ewA, chA[:NF], table_d[0:HALF, :])

                # permute-fold accB into accA (reads bounce written above)
                for off in range(0, SLICE, CMAX):
                    w = min(CMAX, SLICE - off)
                    ws = w // 128
                    msg = msgp.tile([128, CMAX // 128, HID], f32, tag="msg")
                    nc.gpsimd.dma_gather(
                        msg[:, :ws, :], bounce_d[:],
                        rho[:, off // 16:(off + w) // 16],
                        w, w, HID, single_packet=False, queue_num=nextq())
                    nc.vector.tensor_add(
                        accA[:, off // 128:off // 128 + ws, :],
                        accA[:, off // 128:off // 128 + ws, :], msg[:, :ws, :])

                if NF - 1 in batches:
                    emit_tail(batches[NF - 1])
                for k in range(NF, len(chA)):
                    emit_chunks(accA, idxA, ewA, chA[k:k + 1],
                                table_d[0:HALF, :])
                    if k in batches:
                        emit_tail(batches[k])


            nc.sync.dma_start(out_d[:], ostag[:])

    nc.compile()
    return nc


_CACHE = {}


def kernel(x, edge_index, edge_weight, W1, b1, W2, b2, W3, b3, Wlin, blin):
    from concourse.bass_utils import run_bass_kernel_spmd

    x = np.asarray(x, dtype=np.float32)
    assert x.shape == (N, IN_DIM) and np.asarray(edge_index).shape == (2, E)

    key = hash(np.asarray(edge_index).tobytes())
    if key not in _CACHE:
        plan, in_maps = _prep(x, edge_index, edge_weight)
        nc = _build(plan, W1, b1, W2, b2, W3, b3, Wlin, blin)
        _CACHE[key] = (plan, nc)
    else:
        plan, nc = _CACHE[key]
        _, in_maps = _prep(x, edge_index, edge_weight)

    Wlin = np.asarray(Wlin, dtype=np.float32)
    shared = {
        "W1": np.asarray(W1, np.float16), "W2": np.asarray(W2, np.float16),
        "W3": np.concatenate([np.zeros((HID, HID), np.float16), np.asarray(W3, np.float16)], axis=0),
        "Wl12": np.ascontiguousarray(Wlin[0:128]).astype(np.float16),
        "Wl3": np.ascontiguousarray(Wlin[128:192]).astype(np.float16),
        "bias": np.tile(np.concatenate([np.asarray(b, np.float32) for b in (b1, b2, b3)])[None, :], (128, 1)).astype(np.float16),
        "blin": np.tile(np.asarray(blin, np.float32)[None, :], (128, 1)),
    }
    for im in in_maps:
        im.update(shared)

    res = run_bass_kernel_spmd(nc, in_maps, core_ids=list(range(M)))
    kernel._last_results = res
    kernel._last_in_maps = in_maps
    kernel._last_nc = nc

    posA = plan["posA"]
    out = np.empty((N, OUT_DIM), np.float32)
    for c in range(M):
        oc = res.results[c]["out"]  # [128, SLOTS, OUT]
        q = posA[c * NPC:(c + 1) * NPC]
        out[c * NPC:(c + 1) * NPC] = oc[q % 128, q // 128, :]
    return out


# revision 39
# speedup vs baseline: 1.0188x; 1.0188x over previous
"""Trainium2 Bass kernel for a 3-layer GCN (JKNet, mode='cat') — 8-core SPMD.

Strategy (dst-sharded graph parallelism):
  - Nodes are partitioned across 8 cores (6250 each, padded to 6272 = 49*128).
    Each core owns all edges whose destination lands in its range.
  - Per layer: each core computes its slice of h @ W, an AllGather builds the
    full 50176x64 f32 gather table in DRAM, then the core gathers hW[src] for
    its edges with GPSIMD dma_gather (4 SWDGE queues), scales by edge weight
    on DVE and accumulates into an SBUF accumulator using a "rounds" schedule:
    nodes are degree-sorted so round s covers a dense prefix of node slots,
    making the accumulate a plain strided DVE add (no scatter anywhere).
  - dma_gather indices are int16, so the table is split in two halves
    (rows < 25088 / >= 25088, i.e. src cores 0-3 vs 4-7). Each half gets its
    own degree-sort permutation and accumulator; the second accumulator is
    reconciled through a DRAM bounce + local permute-gather.
  - Large gather chunks (4096 rows) amortize the ~1us fixed SWDGE cost per
    dma_gather instruction. Messages are scaled into fp16 and accumulated in
    fp16 (2x DVE rate); the gather table stays f32 (256B-row requirement).
  - Bias + ReLU on DVE/ACT, PE transpose (fp16) produces h^T for the next
    layer's matmul and for the final JumpingKnowledge concat matmul.

Self-contained: hardcodes the problem geometry (N=50000, E=800000, 128->64,
3 layers, out 40) but computes all data-dependent schedules from the inputs.
"""

import sys

sys.path.insert(0, "/opt/trn_rl_repo")

import numpy as np

N = 50000
E = 800000
IN_DIM = 128
HID = 64
OUT_DIM = 40
M = 8               # cores
NPC = N // M        # 6250 nodes per core
SLOTS = 49          # ceil(6250/128)
SLICE = SLOTS * 128  # 6272 padded rows per core slice
TABLE_ROWS = M * SLICE  # 50176
HALF = 4 * SLICE    # 25088 (int16-safe boundary; src cores 0-3 vs 4-7)
CMAX = 896          # max indices per dma_gather instruction
NQ = 4              # SWDGE queues


def _wrap16(a):
    """Flat [L] -> [128, L//16] int16, index j at partition j%16, slot j//16,
    replicated across the 8 GPSIMD core groups."""
    L = a.shape[0]
    return np.tile(a.reshape(L // 16, 16).T, (8, 1)).astype(np.int16)


def _wrap128(a):
    """Flat [L] -> [128, L//128], position j at partition j%128, slot j//128."""
    L = a.shape[0]
    return np.ascontiguousarray(a.reshape(L // 128, 128).T)


def _rowof(q):
    """acc position q -> wrapped DRAM row index (partition-major layout)."""
    return (q % 128) * SLOTS + q // 128


def _ranks_within(p):
    """For int array p, rank of each element among equal values (stable)."""
    order = np.argsort(p, kind="stable")
    ps = p[order]
    starts = np.r_[0, np.nonzero(np.diff(ps))[0] + 1]
    counts = np.diff(np.r_[starts, len(ps)])
    r_sorted = np.arange(len(ps)) - np.repeat(starts, counts)
    r = np.empty_like(r_sorted)
    r[order] = r_sorted
    return r


def _build_system(pos_of_dst, table_row_of_src, ew, max_rounds_widths, chunks):
    """Build flat (idx, ew) arrays for one round-system of one core.

    pos_of_dst: per-edge acc position of the destination node (degree-sorted).
    table_row_of_src: per-edge gather index (already half-relative).
    max_rounds_widths: global per-round padded widths W_s (list, multiple of 128).
    chunks: output of _chunk_plan; positions past each chunk's real count get
    index -1 so the gather ucode skips them (num_idxs_reg).
    Returns (idx_flat int64, ew_flat f32).
    """
    roundoff = np.r_[0, np.cumsum(max_rounds_widths)]
    L = int(roundoff[-1])
    idx_flat = np.zeros(L, np.int64)
    ew_flat = np.zeros(L, np.float32)
    r = _ranks_within(pos_of_dst)
    flatpos = roundoff[r] + pos_of_dst
    idx_flat[flatpos] = table_row_of_src
    ew_flat[flatpos] = ew
    for (off, w, _segs, cnt) in chunks:
        idx_flat[off + cnt:off + w] = -1
    return idx_flat, ew_flat


def _plan_rounds(deg_by_core):
    """deg_by_core: [M, NPC] degree of each node (in its own sort order, desc).
    Returns (padded round widths, real max counts), widths multiple of 128."""
    smax = int(max(d[0] for d in deg_by_core)) if len(deg_by_core) else 0
    widths, reals = [], []
    for s in range(smax):
        n_s = max(int((d > s).sum()) for d in deg_by_core)
        if n_s == 0:
            break
        widths.append(((n_s + 127) // 128) * 128)
        reals.append(n_s)
    return widths, reals


def _chunk_plan(widths, reals, total_pad128):
    """Split the concatenated padded rounds into gather chunks of at most
    CMAX, never crossing a round boundary (so per-round trailing padding is
    trailing within its chunk and can be skipped via num_idxs_reg).
    Returns list of (off, w, [(msg_slot0, acc_slot0, nslots)], real_cnt)."""
    roundoff = np.r_[0, np.cumsum(widths)].astype(np.int64)
    assert int(roundoff[-1]) == total_pad128
    chunks = []
    for s, w_s in enumerate(widths):
        start = int(roundoff[s])
        off = start
        while off < start + w_s:
            w = min(CMAX, start + w_s - off)
            cnt = max(0, min(reals[s] - (off - start), w))
            assert cnt > 0
            chunks.append((off, w, [(0, (off - start) // 128, w // 128)], cnt))
            off += w
    return chunks


def _prep(x, edge_index, edge_weight):
    """All host-side index prep. Returns (plan dict, per-core input maps)."""
    src = np.asarray(edge_index[0], dtype=np.int64)
    dst = np.asarray(edge_index[1], dtype=np.int64)
    ew = np.asarray(edge_weight, dtype=np.float32)
    x = np.asarray(x, dtype=np.float32)

    dcore = dst // NPC
    dloc = dst - dcore * NPC
    score = src // NPC
    is_own = np.zeros_like(score, dtype=bool)
    is_a = (score <= 3) & ~is_own
    is_b = (score > 3) & ~is_own

    # per-core degree sorts for the three systems
    posA = np.empty(N, np.int64)   # node -> acc/table position (A order)
    posB = np.empty(N, np.int64)
    posO = np.empty(N, np.int64)
    piA_all = []
    degA_sorted, degB_sorted, degO_sorted = [], [], []
    for c in range(M):
        mask = dcore == c
        la = dloc[mask & is_a]
        lb = dloc[mask & is_b]
        lo = dloc[mask & is_own]
        degA = np.bincount(la, minlength=NPC)
        degB = np.bincount(lb, minlength=NPC)
        degO = np.bincount(lo, minlength=NPC)
        piA = np.argsort(-degA, kind="stable")
        piB = np.argsort(-degB, kind="stable")
        piO = np.argsort(-degO, kind="stable")
        pA = np.empty(NPC, np.int64); pA[piA] = np.arange(NPC)
        pB = np.empty(NPC, np.int64); pB[piB] = np.arange(NPC)
        pO = np.empty(NPC, np.int64); pO[piO] = np.arange(NPC)
        posA[c * NPC:(c + 1) * NPC] = pA
        posB[c * NPC:(c + 1) * NPC] = pB
        posO[c * NPC:(c + 1) * NPC] = pO
        piA_all.append(piA)
        degA_sorted.append(degA[piA])
        degB_sorted.append(degB[piB])
        degO_sorted.append(degO[piO])

    widthsA, realsA = _plan_rounds(degA_sorted)
    widthsB, realsB = _plan_rounds(degB_sorted)
    LA = int(np.sum(widthsA))
    LB = int(np.sum(widthsB))
    LO = 0
    chunksA = _chunk_plan(widthsA, realsA, LA)
    chunksB = _chunk_plan(widthsB, realsB, LB)
    chunksO = []

    # global table row of a node (wrapped within its owner's slice)
    table_row = (np.arange(N) // NPC) * SLICE + _rowof(posA)

    in_maps = []
    for c in range(M):
        mask = dcore == c
        mA = mask & is_a
        mB = mask & is_b
        mO = mask & is_own
        idxA, ewA = _build_system(posA[dst[mA]] , table_row[src[mA]], ew[mA], widthsA, chunksA)
        idxB, ewB = _build_system(posB[dst[mB]], table_row[src[mB]] - HALF, ew[mB], widthsB, chunksB)
        assert idxA.max(initial=0) < HALF and idxB.max(initial=0) < HALF

        # permute maps: A-position q -> wrapped bounce row of the same node's
        # B/O-position. Pad positions (>= NPC) point at an always-zero row.
        piA = piA_all[c]
        rho = np.full(SLICE, NPC, np.int64)
        rho[:NPC] = posB[c * NPC + piA]
        rho_rows = _rowof(rho)

        # x slice, transposed, in A order (pad columns zero)
        xT = np.zeros((IN_DIM, SLICE), np.float16)
        xT[:, :NPC] = x[c * NPC + piA, :].T

        in_maps.append({
            "xT": xT,
            "idxA": _wrap16(idxA), "ewA": _wrap128(ewA),
            "idxB": _wrap16(idxB), "ewB": _wrap128(ewB),
            "rho": _wrap16(rho_rows),
        })

    plan = {
        "LA": LA, "LB": LB, "LO": LO,
        "chunksA": chunksA, "chunksB": chunksB, "chunksO": chunksO,
        "widthsA": widthsA,
        "posA": posA,
    }
    return plan, in_maps


def _build(plan, W1, b1, W2, b2, W3, b3, Wlin, blin):
    import concourse.bacc as bacc
    import concourse.mybir as mybir
    import concourse.tile as tile

    LA, LB, LO = plan["LA"], plan["LB"], plan["LO"]
    f32 = mybir.dt.float32
    f16 = mybir.dt.float16
    i16 = mybir.dt.int16

    nc = bacc.Bacc("TRN2", target_bir_lowering=False, debug=False,
                   num_devices=M, num_swdge_queues=NQ)

    # ---- I/O ----
    xT_d = nc.dram_tensor("xT", [IN_DIM, SLICE], f16, kind="ExternalInput")
    idxA_d = nc.dram_tensor("idxA", [128, LA // 16], i16, kind="ExternalInput")
    ewA_d = nc.dram_tensor("ewA", [128, LA // 128], f32, kind="ExternalInput")
    idxB_d = nc.dram_tensor("idxB", [128, LB // 16], i16, kind="ExternalInput")
    ewB_d = nc.dram_tensor("ewB", [128, LB // 128], f32, kind="ExternalInput")
    rho_d = nc.dram_tensor("rho", [128, SLICE // 16], i16, kind="ExternalInput")

    W1_d = nc.dram_tensor("W1", [IN_DIM, HID], f16, kind="ExternalInput")
    W2_d = nc.dram_tensor("W2", [HID, HID], f16, kind="ExternalInput")
    W3_d = nc.dram_tensor("W3", [128, HID], f16, kind="ExternalInput")  # rows 64-127 hold W3
    Wl12_d = nc.dram_tensor("Wl12", [128, OUT_DIM], f16, kind="ExternalInput")
    Wl3_d = nc.dram_tensor("Wl3", [HID, OUT_DIM], f16, kind="ExternalInput")
    bias_d = nc.dram_tensor("bias", [128, 3 * HID], f16, kind="ExternalInput")
    blin_d = nc.dram_tensor("blin", [128, OUT_DIM], f32, kind="ExternalInput")
    out_d = nc.dram_tensor("out", [128, SLOTS, OUT_DIM], f32, kind="ExternalOutput")

    # internal DRAM
    slice_d = nc.dram_tensor("slice_hw", [128, SLOTS, HID], f32)
    table_d = nc.dram_tensor("table", [TABLE_ROWS, HID], f32, addr_space="Shared")
    bounce_d = nc.dram_tensor("bounce", [SLICE, HID], f32)


    qctr = [0]

    def nextq():
        q = qctr[0] % NQ
        qctr[0] += 1
        return q

    with tile.TileContext(nc) as tc:
        with (
            tc.tile_pool(name="const", bufs=1) as constp,
            tc.tile_pool(name="acc", bufs=1) as accp,
            tc.tile_pool(name="ht", bufs=1) as htp,
            tc.tile_pool(name="stag", bufs=1) as stagp,
            tc.tile_pool(name="msg", bufs=24) as msgp,
            tc.tile_pool(name="msgw", bufs=24) as msgwp,
            tc.tile_pool(name="warm", bufs=NQ) as warmp,
            tc.tile_pool(name="ps", bufs=3, space="PSUM") as psp,
            tc.tile_pool(name="pso", bufs=2, space="PSUM") as psop,
        ):
            # ---- load constants ----
            xT = constp.tile([IN_DIM, SLICE], f16)
            idxA = constp.tile([128, LA // 16], i16)
            ewA = constp.tile([128, LA // 128], f32)
            idxB = constp.tile([128, LB // 16], i16)
            ewB = constp.tile([128, LB // 128], f32)
            rho = constp.tile([128, SLICE // 16], i16)

            W1t = constp.tile([IN_DIM, HID], f16)
            W2t = constp.tile([HID, HID], f16)
            W3t = constp.tile([128, HID], f16)  # W3 lives in partitions 64-127
            Wl12t = constp.tile([128, OUT_DIM], f16)
            Wl3t = constp.tile([HID, OUT_DIM], f16)
            biast = constp.tile([128, 3 * HID], f16)
            blint = constp.tile([128, OUT_DIM], f32)
            ident = constp.tile([128, 128], f16)

            for k in range(0, SLOTS, 7):
                cs = slice(k * 128, (k + 7) * 128)
                nc.sync.dma_start(xT[:, cs], xT_d[:, cs])
            for t, d in ((idxA, idxA_d), (ewA, ewA_d),
                         (idxB, idxB_d), (ewB, ewB_d), (rho, rho_d),
                         (W1t, W1_d), (W2t, W2_d), (Wl12t, Wl12_d),
                         (Wl3t, Wl3_d),
                         (biast, bias_d), (blint, blin_d)):
                nc.sync.dma_start(t[:], d[:])
            nc.sync.dma_start(W3t[:], W3_d[:])
            from concourse.masks import make_identity
            make_identity(nc, ident[:])

            for _ in range(24):
                wt = msgp.tile([128, CMAX // 128, HID], f32, tag="msg")
                nc.vector.memset(wt[:], 0.0)

            h12T = htp.tile([128, SLICE], f16)   # rows 0-63: h1^T, 64-127: h2^T
            h3T = htp.tile([HID, SLICE], f16)

            relu = mybir.ActivationFunctionType.Relu
            copyf = mybir.ActivationFunctionType.Copy
            rfull = nc.gpsimd.to_reg(CMAX)

            # ---- layer-1 input matmuls: slice of x @ W1 ----
            stag = stagp.tile([128, SLOTS, HID], f32, tag="stag")
            for m in range(SLOTS):
                ps = psp.tile([128, HID], f32, tag="mm")
                nc.tensor.matmul(ps[:], xT[:, m * 128:(m + 1) * 128], W1t[:],
                                 start=True, stop=True)
                nc.scalar.activation(stag[:, m, :], ps[:], copyf)
            nc.sync.dma_start(slice_d[:], stag[:])

            ostag = stagp.tile([128, SLOTS, OUT_DIM], f32, tag="ostag")

            for layer in range(3):
                # ---- AllGather the table for this layer ----
                nc.gpsimd.collective_compute(
                    "AllGather", mybir.AluOpType.bypass,
                    replica_groups=[list(range(M))],
                    ins=[slice_d[:]], outs=[table_d[:]],
                )

                accA = accp.tile([128, SLOTS, HID], f16, tag="accA")
                accB = accp.tile([128, SLOTS, HID], f16, tag="accB")
                nc.vector.memset(accA[:], 0.0)
                nc.vector.memset(accB[:], 0.0)
                if True:
                    # tiny warmup gathers on each queue while the AllGather
                    # runs, so post-collective DGE state reload happens off
                    # the critical path (results are discarded)
                    for _ in range(NQ):
                        wmsg = warmp.tile([128, 1, HID], f32, tag="wmsg")
                        nc.gpsimd.dma_gather(
                            wmsg[:, 0:1, :], bounce_d[:], rho[:, 0:8],
                            128, 128, HID, single_packet=False,
                            queue_num=nextq())

                if layer < 2:
                    stag = stagp.tile([128, SLOTS, HID], f32, tag="stag")

                def emit_chunks(acc, idx_t, ew_t, chunks, tbl, sp=False):
                    for (off, w, segs, cnt) in chunks:
                        ws = w // 128
                        msg = msgp.tile([128, CMAX // 128, HID], f32, tag="msg")
                        msgw = msgwp.tile([128, CMAX // 128, HID], f16, tag="msgw")
                        nreg = rfull if cnt == CMAX else cnt
                        nc.gpsimd.dma_gather(
                            msg[:, :ws, :], tbl, idx_t[:, off // 16:(off + w) // 16],
                            w, nreg, HID, single_packet=sp, queue_num=nextq())
                        nc.vector.tensor_mul(
                            msgw[:, :ws, :], msg[:, :ws, :],
                            ew_t[:, off // 128:(off + w) // 128]
                            .to_broadcast([128, ws, HID]))
                        for (ms, as_, ns) in segs:
                            nc.vector.tensor_add(
                                acc[:, as_:as_ + ns, :], acc[:, as_:as_ + ns, :],
                                msgw[:, ms:ms + ns, :])

                bslice = biast[:, layer * HID:(layer + 1) * HID]

                def emit_tail(slots_list):
                    # finalize the given acc slots: bias+relu, transpose into
                    # h^T, next-layer matmul, staging write. Runs on PE/ACT/
                    # DVE while GPSIMD keeps gathering remaining rounds.
                    slots_list = sorted(slots_list)
                    ranges = []
                    for m in slots_list:
                        if ranges and ranges[-1][1] == m:
                            ranges[-1][1] = m + 1
                        else:
                            ranges.append([m, m + 1])
                    for (m0, m1) in ranges:
                        r = m1 - m0
                        nc.vector.tensor_add(
                            accA[:, m0:m1, :], accA[:, m0:m1, :],
                            bslice.rearrange("p (s d) -> p s d", s=1)
                            .to_broadcast([128, r, HID]))
                        nc.scalar.activation(accA[:, m0:m1, :],
                                             accA[:, m0:m1, :], relu)
                    for (m0, m1) in ranges:
                        for m in range(m0, m1):
                            pst = psp.tile([HID, 128], f16, tag="tr")
                            nc.tensor.transpose(pst[:], accA[:, m, :], ident[:])
                            sl = slice(m * 128, (m + 1) * 128)
                            if layer == 0:
                                dst_ap = h12T[0:HID, sl]
                            elif layer == 1:
                                dst_ap = h12T[HID:128, sl]
                            else:
                                dst_ap = h3T[:, sl]
                            if m % 2 == 0:
                                nc.vector.tensor_copy(dst_ap, pst[:])
                            else:
                                nc.scalar.activation(dst_ap, pst[:], copyf)
                    for (m0, m1) in ranges:
                        for m in range(m0, m1):
                            sl = slice(m * 128, (m + 1) * 128)
                            if layer == 0:
                                ps = psp.tile([128, HID], f32, tag="mm")
                                nc.tensor.matmul(ps[:], h12T[0:HID, sl], W2t[:],
                                                 start=True, stop=True)
                                nc.scalar.activation(stag[:, m, :], ps[:], copyf)
                            elif layer == 1:
                                ps = psp.tile([128, HID], f32, tag="mm")
                                nc.tensor.matmul(ps[:], h12T[HID:128, sl],
                                                 W3t[HID:128, :],
                                                 start=True, stop=True)
                                nc.scalar.activation(stag[:, m, :], ps[:], copyf)
                            else:
                                pso = psop.tile([128, OUT_DIM], f32, tag="out")
                                nc.tensor.matmul(pso[:], h12T[:, sl],
                                                 Wl12t[:], start=True, stop=False)
                                nc.tensor.matmul(pso[:], h3T[:, sl],
                                                 Wl3t[:], start=False, stop=True)
                                nc.vector.tensor_add(ostag[:, m, :], pso[:],
                                                     blint[:])
                        if layer < 2:
                            nc.sync.dma_start(slice_d[:, m0:m1, :],
                                              stag[:, m0:m1, :])

                # per-slot finalization schedule: slot m is final after the
                # last A-chunk whose rounds still reach it (round widths
                # shrink, so high slots finalize early)
                import numpy as _np
                widthsA = plan["widthsA"]
                chA = plan["chunksA"]
                roundoffA = _np.r_[0, _np.cumsum(widthsA)].astype(_np.int64)
                NF = 14
                batches = {}
                for m in range(SLOTS):
                    ss = [s for s in range(len(widthsA))
                          if widthsA[s] > 128 * m]
                    send = int(roundoffA[max(ss) + 1]) - 1
                    for k, (off, w, _s, _c) in enumerate(chA):
                        if off <= send < off + w:
                            batches.setdefault(max(k, NF - 1), []).append(m)
                            break

                # B system first so its bounce DMA overlaps the A gathers
                emit_chunks(accB, idxB, ewB, plan["chunksB"],
                            table_d[HALF:TABLE_ROWS, :])
                # fp16 acc -> f32 on the idle ACT engine (gather needs 256B
                # rows), then a sync-engine DMA so GPSIMD is never blocked
                accB32 = accp.tile([128, SLOTS, HID], f32, tag="accB32")
                nc.scalar.activation(accB32[:], accB[:], copyf)
                nc.sync.dma_start(
                    bounce_d[:].rearrange("(p s) d -> p s d", p=128), accB32[:])
                emit_chunks(accA, idxA, ewA, chA[:NF], table_d[0:HALF, :])

                # permute-fold accB into accA (reads bounce written above)
                for off in range(0, SLICE, CMAX):
                    w = min(CMAX, SLICE - off)
                    ws = w // 128
                    msg = msgp.tile([128, CMAX // 128, HID], f32, tag="msg")
                    nc.gpsimd.dma_gather(
                        msg[:, :ws, :], bounce_d[:],
                        rho[:, off // 16:(off + w) // 16],
                        w, w, HID, single_packet=False, queue_num=nextq())
                    nc.vector.tensor_add(
                        accA[:, off // 128:off // 128 + ws, :],
                        accA[:, off // 128:off // 128 + ws, :], msg[:, :ws, :])

                if NF - 1 in batches:
                    emit_tail(batches[NF - 1])
                for k in range(NF, len(chA)):
                    emit_chunks(accA, idxA, ewA, chA[k:k + 1],
                                table_d[0:HALF, :])
                    if k in batches:
                        emit_tail(batches[k])


            nc.sync.dma_start(out_d[:], ostag[:])

    nc.compile()
    return nc


_CACHE = {}


def kernel(x, edge_index, edge_weight, W1, b1, W2, b2, W3, b3, Wlin, blin):
    from concourse.bass_utils import run_bass_kernel_spmd

    x = np.asarray(x, dtype=np.float32)
    assert x.shape == (N, IN_DIM) and np.asarray(edge_index).shape == (2, E)

    key = hash(np.asarray(edge_index).tobytes())
    if key not in _CACHE:
        plan, in_maps = _prep(x, edge_index, edge_weight)
        nc = _build(plan, W1, b1, W2, b2, W3, b3, Wlin, blin)
        _CACHE[key] = (plan, nc)
    else:
        plan, nc = _CACHE[key]
        _, in_maps = _prep(x, edge_index, edge_weight)

    Wlin = np.asarray(Wlin, dtype=np.float32)
    shared = {
        "W1": np.asarray(W1, np.float16), "W2": np.asarray(W2, np.float16),
        "W3": np.concatenate([np.zeros((HID, HID), np.float16), np.asarray(W3, np.float16)], axis=0),
        "Wl12": np.ascontiguousarray(Wlin[0:128]).astype(np.float16),
        "Wl3": np.ascontiguousarray(Wlin[128:192]).astype(np.float16),
        "bias": np.tile(np.concatenate([np.asarray(b, np.float32) for b in (b1, b2, b3)])[None, :], (128, 1)).astype(np.float16),
        "blin": np.tile(np.asarray(blin, np.float32)[None, :], (128, 1)),
    }
    for im in in_maps:
        im.update(shared)

    res = run_bass_kernel_spmd(nc, in_maps, core_ids=list(range(M)))
    kernel._last_results = res
    kernel._last_in_maps = in_maps
    kernel._last_nc = nc

    posA = plan["posA"]
    out = np.empty((N, OUT_DIM), np.float32)
    for c in range(M):
        oc = res.results[c]["out"]  # [128, SLOTS, OUT]
        q = posA[c * NPC:(c + 1) * NPC]
        out[c * NPC:(c + 1) * NPC] = oc[q % 128, q // 128, :]
    return out
